# revision 3
# baseline (speedup 1.0000x reference)
"""Trainium2 Bass kernel for nn_Attention (linear attention, no softmax).

Key identity: without softmax, (Q K^T) V = Q (K^T V), so the whole block
collapses to per-batch [C,C] matrices:
    S   = xs^T xs                      [C,C]   (xs = [L,C] tokens)
    At_h = Wk_h^T Wq_h  (= A_h^T)      [C,C]   batch-independent
    B_h  = Wv_h^T Wo_h^T               [C,C]   batch-independent
    Tt_h = S At_h   (= (A_h S)^T)      [C,C]
    G    = sum_h Tt_h^T B_h            [C,C]
    out  = (G^T X) + bias              [C,L]   (X = xs^T, the native x layout)

Sharding: data-parallel over batch, 2 batches per core across 8 cores.
"""

import numpy as np

P = 128
B_FULL, C, W, H = 16, 256, 32, 32
L = W * H  # 1024
NH = 4
NCORES = 8
BPC = B_FULL // NCORES  # batches per core = 2
CT = C // P   # 2 c-tiles
LT = L // P   # 8 L-tiles
NZ = L // 512  # 2 output column chunks

# float32r: fp32 bits, fast PE path (1 cyc/row at N>=256 vs 4 for fp32)
_MM_DTYPE = "float32r"

_CACHE = {}


def _build_program():
    import concourse.bacc as bacc
    import concourse.mybir as mybir
    import concourse.tile as tile

    f32 = mybir.dt.float32
    mmdt = getattr(mybir.dt, _MM_DTYPE)
    AF = mybir.ActivationFunctionType

    nc = bacc.Bacc("TRN2", target_bir_lowering=False, debug=False)

    # matmul-feeding tensors are declared in the matmul dtype end-to-end
    # (the BIR verifier requires f32r inputs to be *produced* as f32r).
    x2d_d = nc.dram_tensor("x2d", [BPC, C, L], mmdt, kind="ExternalInput").ap()
    xs_d = nc.dram_tensor("xs", [BPC, L, C], mmdt, kind="ExternalInput").ap()
    wq_d = nc.dram_tensor("wq", [NH, C, C], mmdt, kind="ExternalInput").ap()
    wk_d = nc.dram_tensor("wk", [NH, C, C], mmdt, kind="ExternalInput").ap()
    wv_d = nc.dram_tensor("wv", [NH, C, C], mmdt, kind="ExternalInput").ap()
    wot_d = nc.dram_tensor("wot", [NH, C, C], mmdt, kind="ExternalInput").ap()
    wob_d = nc.dram_tensor("wob", [C, 1], f32, kind="ExternalInput").ap()
    out_d = nc.dram_tensor("out", [BPC, C, L], f32, kind="ExternalOutput").ap()

    def mm(ps_ap, lhsT_ap, rhs_ap, start, stop):
        nc.tensor.matmul(ps_ap, lhsT_ap, rhs_ap, start=start, stop=stop)

    with tile.TileContext(nc) as tc:
        from contextlib import ExitStack

        with ExitStack() as ctx:
            const = ctx.enter_context(tc.tile_pool(name="const", bufs=1))
            work = ctx.enter_context(tc.tile_pool(name="work", bufs=2))
            zpool = ctx.enter_context(tc.tile_pool(name="zout", bufs=4))
            psum = ctx.enter_context(tc.tile_pool(name="psum", bufs=6, space="PSUM"))

            # ---- load weights: W_sb[:, h*CT+kt, :] = W[h, kt*P:(kt+1)*P, :]
            wq_sb = const.tile([P, NH * CT, C], mmdt, tag="wq")
            wk_sb = const.tile([P, NH * CT, C], mmdt, tag="wk")
            wv_sb = const.tile([P, NH * CT, C], mmdt, tag="wv")
            wot_sb = const.tile([P, NH * CT, C], mmdt, tag="wot")
            for w_sb, w_d in ((wq_sb, wq_d), (wk_sb, wk_d), (wv_sb, wv_d), (wot_sb, wot_d)):
                for h in range(NH):
                    for kt in range(CT):
                        nc.sync.dma_start(
                            w_sb[:, h * CT + kt, :], w_d[h, kt * P:(kt + 1) * P, :]
                        )
            bias_sb = const.tile([P, CT], f32, tag="bias")
            for m in range(CT):
                nc.sync.dma_start(bias_sb[:, m:m + 1], wob_d[m * P:(m + 1) * P, :])

            # ---- At_h = Wk_h^T Wq_h ; B_h = Wv_h^T WoT_h   (once per core)
            # layout [P, row_tile, NH*C] flat: [:, rt, h*C:(h+1)*C]
            at_sb = const.tile([P, CT, NH * C], mmdt, tag="at")
            b_sb = const.tile([P, CT, NH * C], mmdt, tag="b")
            for dst, lhs_w, rhs_w in ((at_sb, wk_sb, wq_sb), (b_sb, wv_sb, wot_sb)):
                for h in range(NH):
                    for m in range(CT):
                        ps = psum.tile([P, 512], mybir.dt.float32, tag="ps")
                        for kt in range(CT):
                            mm(ps[:, :C],
                               lhs_w[:, h * CT + kt, m * P:(m + 1) * P],
                               rhs_w[:, h * CT + kt, :],
                               kt == 0, kt == CT - 1)
                        nc.any.tensor_copy(dst[:, m, h * C:(h + 1) * C], ps[:, :C])

            # ---- per batch
            for b in range(BPC):
                x_sb = work.tile([P, CT, L], mmdt, tag="x")
                xs_sb = work.tile([P, LT, C], mmdt, tag="xs")
                for ct in range(CT):
                    nc.sync.dma_start(x_sb[:, ct, :], x2d_d[b, ct * P:(ct + 1) * P, :])
                for lt in range(LT):
                    nc.sync.dma_start(xs_sb[:, lt, :], xs_d[b, lt * P:(lt + 1) * P, :])

                # S = xs^T xs  [C,C], symmetric; S_sb[:, rt, :] = S[rt*P:(rt+1)*P, :]
                s_sb = work.tile([P, CT, C], mmdt, tag="s")
                for m in range(CT):
                    ps = psum.tile([P, 512], f32, tag="ps")
                    for lt in range(LT):
                        mm(ps[:, :C],
                           xs_sb[:, lt, m * P:(m + 1) * P],
                           xs_sb[:, lt, :],
                           lt == 0, lt == LT - 1)
                    nc.any.tensor_copy(s_sb[:, m, :], ps[:, :C])

                # Tt_h = S At_h ; layout [P, rt, NH*C]
                tt_sb = work.tile([P, CT, NH * C], mmdt, tag="tt")
                for m in range(CT):
                    for hp in range(NH // 2):  # head pairs -> N=512
                        ps = psum.tile([P, 512], f32, tag="ps")
                        for kt in range(CT):
                            mm(ps[:],
                               s_sb[:, kt, m * P:(m + 1) * P],
                               at_sb[:, kt, hp * 512:(hp + 1) * 512],
                               kt == 0, kt == CT - 1)
                        nc.any.tensor_copy(tt_sb[:, m, hp * 512:(hp + 1) * 512], ps[:])

                # G = sum_h Tt_h^T B_h
                g_sb = work.tile([P, CT, C], mmdt, tag="g")
                for m in range(CT):
                    ps = psum.tile([P, 512], f32, tag="ps")
                    i, n_acc = 0, NH * CT
                    for h in range(NH):
                        for kt in range(CT):
                            mm(ps[:, :C],
                               tt_sb[:, kt, h * C + m * P: h * C + (m + 1) * P],
                               b_sb[:, kt, h * C:(h + 1) * C],
                               i == 0, i == n_acc - 1)
                            i += 1
                    nc.any.tensor_copy(g_sb[:, m, :], ps[:, :C])

                # out[b] = G^T X + bias   [C, L]
                for m in range(CT):
                    for nt in range(NZ):
                        ps = psum.tile([P, 512], f32, tag="ps")
                        for kt in range(CT):
                            mm(ps[:],
                               g_sb[:, kt, m * P:(m + 1) * P],
                               x_sb[:, kt, nt * 512:(nt + 1) * 512],
                               kt == 0, kt == CT - 1)
                        zt = zpool.tile([P, 512], f32, tag="z")
                        nc.scalar.activation(
                            zt[:], ps[:], AF.Identity, bias=bias_sb[:, m:m + 1]
                        )
                        nc.sync.dma_start(
                            out_d[b, m * P:(m + 1) * P, nt * 512:(nt + 1) * 512], zt[:]
                        )

    nc.compile()
    return nc


def _get_program():
    if "nc" not in _CACHE:
        _CACHE["nc"] = _build_program()
    return _CACHE["nc"]


def _prep_inputs(x, Wq, Wk, Wv, Wo_w, Wo_b):
    x = np.ascontiguousarray(np.asarray(x, dtype=np.float32))
    X = x.reshape(B_FULL, C, L)                                    # [b, C, L]
    XS = np.ascontiguousarray(X.transpose(0, 2, 1))                # [b, L, C]
    WoT = np.ascontiguousarray(
        np.asarray(Wo_w, dtype=np.float32).T
    ).reshape(NH, C, C)                                            # WoT[h] = Wo_h^T
    common = {
        "wq": np.ascontiguousarray(np.asarray(Wq, dtype=np.float32)),
        "wk": np.ascontiguousarray(np.asarray(Wk, dtype=np.float32)),
        "wv": np.ascontiguousarray(np.asarray(Wv, dtype=np.float32)),
        "wot": WoT,
        "wob": np.ascontiguousarray(np.asarray(Wo_b, dtype=np.float32).reshape(C, 1)),
    }
    in_maps = []
    for i in range(NCORES):
        in_maps.append({
            "x2d": np.ascontiguousarray(X[i * BPC:(i + 1) * BPC]),
            "xs": np.ascontiguousarray(XS[i * BPC:(i + 1) * BPC]),
            **common,
        })
    return in_maps


def run_sharded(inputs, trace=False, trace_cores=None):
    """Run the SPMD kernel; returns (full_output, BassKernelResults)."""
    from concourse.bass_utils import run_bass_kernel_spmd

    in_maps = _prep_inputs(**inputs)
    nc = _get_program()
    res = run_bass_kernel_spmd(
        nc, in_maps, core_ids=list(range(NCORES)),
        trace=trace, trace_cores=trace_cores,
    )
    out = np.concatenate([res.results[i]["out"] for i in range(NCORES)], axis=0)
    return out.reshape(B_FULL, C, W, H).astype(np.float32), res


def kernel(x, Wq, Wk, Wv, Wo_w, Wo_b):
    out, _ = run_sharded(
        {"x": x, "Wq": Wq, "Wk": Wk, "Wv": Wv, "Wo_w": Wo_w, "Wo_b": Wo_b}
    )
    return out
